# revision 26
# baseline (speedup 1.0000x reference)
"""BLSTM GermEval kernel for 8x TRN2 NeuronCores.

Strategy: data-parallel over batch (B=64 -> 8 rows/core). Each core runs the
full network on its slice: embedding gather, per-layer input projections
(GEMM, bf16 with f32 accum), two BLSTM layers (256-step recurrences),
dense + softmax. The recurrence is fully unrolled (no hardware loop): all
DMAs are static/prefetchable and the Tile scheduler can overlap adjacent
steps and hide the input-projection phase under the recurrence.

Key tricks:
 - backward direction runs time-reversed with the same (t < len) masks, so no
   length-dependent sequence reversal is ever materialized; fw/bw each use
   their own token ordering ("processing order") end-to-end, and flipped
   h-history copies bridge the two orderings between layers.
 - sequence masking is folded into the precomputed input projections as
   per-(token,gate) biases: masked rows get sigmoid(i)=0, sigmoid(f)=1,
   sigmoid(o)=0, which freezes c and zeroes h exactly (exact even in bf16:
   sigmoid(+-41) rounds to exactly 1/0).
 - fw and bw are packed on PSUM partitions 0-7 / 32-39 of shared [40, 512]
   gate banks (out base partition must be 32-aligned), so each gate needs
   ONE activation / DVE op for both directions; gate column order is
   (j, i, f, o), reordered host-side.
 - recurrent matmul streams W_h as the moving operand (stationary lhsT =
   h^T slice of the history buffer); the id40z scatter-matmul seeds all 40
   PSUM partitions with the precomputed gate biases (zeroing idle lanes) in
   one instruction per gate.
 - h^T is written straight into the per-(dir,k) history buffers by PE
   transposes of [40,128] h chunks (both directions in one transpose), and
   those buffers double as next-step matmul stationaries and next-layer
   gemm inputs.
 - the whole c/h elementwise chain runs in bf16 (DVE 2x mode).
"""

import numpy as np
from contextlib import ExitStack

V, E, H, L, C, B, S = 50000, 300, 512, 2, 25, 64, 256
NCORES = 8
BL = B // NCORES          # batch rows per core
T = S * BL                # tokens per core
G4 = 4 * H                # gate width
MASK_BIG = 40.0

_compiled = None


def _patch_bass():
    """Work around this walrus build's 1-sync-wait-per-instruction limit."""
    import concourse.bass as bassmod
    import concourse.mybir as mybir

    if getattr(bassmod, "_blstm_patched", False):
        return
    bassmod._blstm_patched = True

    _orig_dma_reset = bassmod.BassGpSimd.dma_reset

    def _chunked_dma_reset(self, semaphore_range=None):
        if semaphore_range is None:
            semaphore_range = self.bass._kernel_sem_range
        last = None
        for s in range(semaphore_range.start, semaphore_range.stop, 16):
            last = self.drain(
                semaphore_range=range(s, min(s + 16, semaphore_range.stop))
            )
        return last

    bassmod.BassGpSimd.dma_reset = _chunked_dma_reset


def _fix_sync_waits(nc):
    """Move excess sem-waits onto injected NoOps (walrus wait-slot limit)."""
    import concourse.mybir as mybir

    nid = 0
    for f in nc.m.functions:
        for blk in f.blocks:
            insts = list(blk.instructions)
            out, changed = [], False
            for inst in insts:
                si = inst.sync_info
                if si is not None and len(si.on_wait) > 1:
                    waits = list(si.on_wait)
                    for w in waits[1:]:
                        nid += 1
                        nop = mybir.InstNoOp(name=f"waitfix-{nid}", ins=[], outs=[])
                        nop.engine = inst.engine
                        nop.sync_info = mybir.SyncInfo(on_wait=[w], on_update=[])
                        out.append(nop)
                    si.on_wait = waits[:1]
                    changed = True
                out.append(inst)
            if changed:
                blk.instructions = out


def _build():
    import concourse.bass as bass
    import concourse.mybir as mybir
    import concourse.tile as tile
    from concourse.bass import ds
    from concourse.masks import make_identity

    _patch_bass()
    f32 = mybir.dt.float32
    bf16 = mybir.dt.bfloat16
    i32 = mybir.dt.int32
    AF = mybir.ActivationFunctionType
    OP = mybir.AluOpType

    nc = bass.Bass()
    emb_d = nc.dram_tensor("emb", [V, E], f32, kind="ExternalInput")
    ids_d = nc.dram_tensor("ids2", [2, T], i32, kind="ExternalInput")
    mb_d = nc.dram_tensor("mb2", [2, T, 4], f32, kind="ExternalInput")
    w0_d = nc.dram_tensor("w0", [2, E + H, G4], f32, kind="ExternalInput")
    b0_d = nc.dram_tensor("b0", [2, G4], f32, kind="ExternalInput")
    w1_d = nc.dram_tensor("w1", [2, 2 * H + H, G4], f32, kind="ExternalInput")
    b1_d = nc.dram_tensor("b1", [2, G4], f32, kind="ExternalInput")
    wd_d = nc.dram_tensor("wd", [2 * H, C], f32, kind="ExternalInput")
    bd_d = nc.dram_tensor("bd", [C], f32, kind="ExternalInput")
    out_d = nc.dram_tensor("out", [T, C], f32, kind="ExternalOutput")

    import os
    _BI = int(os.environ.get("BLSTM_BISECT", "9"))
    _V2 = os.environ.get("BLSTM_V2", "1") == "1"
    NT = T // 128           # 16 token tiles
    KX = [128, 128, E - 256]  # X^T K-chunks (300)
    HCOLS = 32 * (S + 1)    # h^T history: 32 cols per step (4 chunks x 8)

    with tile.TileContext(nc) as tc, ExitStack() as st:
        persist = st.enter_context(tc.tile_pool(name="persist", bufs=1))
        dram = st.enter_context(tc.tile_pool(name="dram", bufs=1, space="DRAM"))

        id8 = persist.tile([BL, BL], bf16)
        make_identity(nc, id8[:])
        ones1 = persist.tile([1, 128], bf16)
        nc.vector.memset(ones1[:], 1.0)
        id128 = persist.tile([128, 128], bf16)
        make_identity(nc, id128[:])

        # per-(dir) small constants
        mb_t = [persist.tile([128, T // 128, 4], f32, name=f"mb{d}") for d in range(2)]
        for d in range(2):
            nc.sync.dma_start(mb_t[d][:],
                              mb_d[d].rearrange("(m p) g -> p m g", p=128))
        b0_t = [persist.tile([1, G4], bf16, name=f"b0{d}") for d in range(2)]
        b1_t = [persist.tile([1, G4], bf16, name=f"b1{d}") for d in range(2)]
        for d in range(2):
            nc.gpsimd.dma_start(b0_t[d][:], b0_d[d].rearrange("(o g) -> o g", o=1))
            nc.gpsimd.dma_start(b1_t[d][:], b1_d[d].rearrange("(o g) -> o g", o=1))

        gx0 = [dram.tile([S, BL, G4], bf16, name=f"gx0_{d}") for d in range(2)]
        gx1 = [dram.tile([S, BL, G4], bf16, name=f"gx1_{d}") for d in range(2)]

        def gemm_gates(dst, lhsT_chunks, rhs_chunks, b_tile, mbv, psum, epil):
            """dst = (gx_tile, d): gate pre-acts + b + maskbias, token-tiled."""
            gxt, dd = dst
            dstv = gxt[dd][:].rearrange("s b g -> (s b) g")
            for m in range(NT):
                gtile = epil.tile([128, G4], bf16, tag="gemm_out")
                for n in range(4):
                    pb = psum.tile([128, 512], f32, tag="gemm_ps")
                    nc.tensor.matmul(
                        out=pb[:], lhsT=ones1[:],
                        rhs=b_tile[:, 512 * n: 512 * n + 512],
                        start=True, stop=False)
                    nk = len(lhsT_chunks)
                    for k in range(nk):
                        nc.tensor.matmul(
                            out=pb[:],
                            lhsT=lhsT_chunks[k](m),
                            rhs=rhs_chunks[k][:, 512 * n: 512 * n + 512],
                            start=False,
                            stop=(k == nk - 1),
                        )
                    nc.scalar.activation(
                        out=gtile[:, 512 * n: 512 * n + 512], in_=pb[:],
                        func=AF.Identity, bias=mbv[:, m, n: n + 1])
                nc.sync.dma_start(dstv[128 * m: 128 * m + 128, :], gtile[:])

        # ---------------- phase 1: gather + layer-0 input projections --------
        with tc.tile_pool(name="ph1", bufs=1) as ph1, \
             tc.tile_pool(name="ph1w", bufs=3) as ph1w, \
             tc.tile_pool(name="ph1p", bufs=4, space="PSUM") as ph1p:
            xT = [[ph1.tile([KX[k], T], bf16, name=f"xT{d}_{k}") for k in range(3)]
                  for d in range(2)]
            wx0 = [[ph1.tile([KX[k], G4], bf16, name=f"wx0{d}_{k}") for k in range(3)]
                   for d in range(2)]
            for d in range(2):
                for k in range(3):
                    o = 128 * k
                    nc.gpsimd.dma_start(wx0[d][k][:], w0_d[d, o:o + KX[k], :])
            for d in range(2):
                for m in range(NT):
                    idx = ph1w.tile([128, 1], i32, tag="idx")
                    nc.sync.dma_start(
                        idx[:], ids_d[d, 128 * m: 128 * m + 128].rearrange("(p o) -> p o", o=1))
                    xg = ph1w.tile([128, E], f32, tag="xg")
                    nc.gpsimd.indirect_dma_start(
                        out=xg[:], out_offset=None, in_=emb_d[:],
                        in_offset=bass.IndirectOffsetOnAxis(ap=idx[:, 0:1], axis=0))
                    xgb = ph1w.tile([128, E], bf16, tag="xgb")
                    nc.vector.tensor_copy(out=xgb[:], in_=xg[:])
                    for k in range(3):
                        pt = ph1p.tile([KX[k], 128], bf16, tag="xtp")
                        nc.tensor.transpose(
                            out=pt[:], in_=xgb[:, 128 * k: 128 * k + KX[k]],
                            identity=id128[:])
                        nc.vector.tensor_copy(
                            out=xT[d][k][:, 128 * m: 128 * m + 128], in_=pt[:])
            for d in range(2):
                gemm_gates(
                    (gx0, d),
                    [(lambda m, _t=xT[d][k]: _t[:, 128 * m: 128 * m + 128])
                     for k in range(3)],
                    wx0[d], b0_t[d], mb_t[d], ph1p, ph1w)

        # ---------------- LSTM pass v2: fw+bw packed on partitions ------------
        # gate column order is (j, i, f, o) — host packs W accordingly.
        # PSUM gate banks [40, 512]: fw rows 0-7, bw rows 32-39 (base-32 rule).
        # h history is written transposed straight into HT[d][k], which doubles
        # as the next step's lhsT and the next layer's gemm input.
        id40 = persist.tile([40, 40], bf16)
        make_identity(nc, id40[:])
        # id40z scatters fw/bw gate biases (rows 0-7 / 32-39 of gb) onto the
        # same out partitions and zeroes partitions 8-31 in one matmul.
        id40z = persist.tile([40, 40], bf16)
        nc.vector.memset(id40z[:], 0.0)
        nc.vector.tensor_copy(out=id40z[0:8, 0:8], in_=id8[:])
        nc.vector.tensor_copy(out=id40z[32:40, 32:40], in_=id8[:])

        def lstm_pass_v2(gx, wh, HT, tag):
            c_t = persist.tile([40, H], bf16, name=f"c{tag}")
            nc.vector.memset(c_t[:], 0.0)
            for d in range(2):
                for k in range(4):
                    nc.vector.memset(HT[d][k][:, 0:8], 0.0)

            with tc.tile_pool(name=f"lp{tag}", bufs=2) as lp, \
                 tc.tile_pool(name=f"lpp{tag}", bufs=1, space="PSUM") as lpp:
                pg = [lpp.tile([40, 512], f32, name=f"pg{tag}{n}") for n in range(4)]
                pt = lpp.tile([128, 160], bf16, name=f"pt{tag}")
                gxv = [gx[d][:].rearrange("s b g -> (s b) g") for d in range(2)]
                gbP = [lp.tile([40, G4], bf16, name=f"gb{j}", tag=f"gb{j}",
                               bufs=1) for j in range(4)]
                for j in range(4):    # rows 8-31 stay zero forever
                    nc.vector.memset(gbP[j][:], 0.0)

                def load_gx(t):
                    so = t * BL
                    gb = gbP[t % 4]
                    nc.sync.dma_start(gb[0:8, :], gxv[0][so:so + BL, :])
                    nc.scalar.dma_start(gb[32:40, :], gxv[1][so:so + BL, :])

                def gx_add(t, banks):
                    """Seed pg[n] with this step's gate biases (own PE group)."""
                    gb = gbP[t % 4]
                    for n in banks:
                        nc.tensor.matmul(
                            out=pg[n][:], lhsT=id40z[:],
                            rhs=gb[:, 512 * n: 512 * n + 512],
                            start=True, stop=True, skip_group_check=True)

                def body(t):
                    if t + 1 < S:
                        load_gx(t + 1)
                    act = {}
                    for n in range(4):          # j, i, f, o
                        # interleave fw (PE col-group 0) and bw (col-group 1)
                        # so their weight streams overlap on the array
                        for k in range(4):
                            for d, pb in ((0, 0), (1, 32)):
                                nc.tensor.matmul(
                                    out=pg[n][pb:pb + 8, :],
                                    lhsT=HT[d][k][:, 8 * t: 8 * t + 8],
                                    rhs=wh[d][k][:, 512 * n: 512 * n + 512],
                                    start=False,
                                    stop=(k == 3),
                                    skip_group_check=True)
                        a = lp.tile([40, 512], bf16, tag=f"act{n}")
                        nc.scalar.activation(
                            out=a[:], in_=pg[n][:],
                            func=AF.Tanh if n == 0 else AF.Sigmoid)
                        act[n] = a
                    # next step's gate-bias seeds run while this step's tail
                    # (c-chain, transposes) is still in flight, keeping PE fed
                    if t + 1 < S:
                        gx_add(t + 1, (0, 1, 2))
                    q = lp.tile([40, H], bf16, tag="q")
                    nc.vector.tensor_mul(out=q[:], in0=act[1][:], in1=act[0][:])
                    p = lp.tile([40, H], bf16, tag="p")
                    nc.vector.tensor_mul(out=p[:], in0=c_t[:], in1=act[2][:])
                    nc.vector.tensor_add(out=c_t[:], in0=q[:], in1=p[:])
                    tc_ = lp.tile([40, H], bf16, tag="tc")
                    nc.scalar.activation(out=tc_[:], in_=c_t[:], func=AF.Tanh)
                    hn = lp.tile([40, H], bf16, tag="hn")
                    nc.vector.tensor_mul(out=hn[:], in0=act[3][:], in1=tc_[:])
                    for k in range(4):
                        nc.tensor.transpose(
                            out=pt[:, 40 * k: 40 * k + 40],
                            in_=hn[:, 128 * k: 128 * k + 128],
                            identity=id40[:])
                        nc.vector.tensor_copy(
                            out=HT[0][k][:, 8 * (t + 1): 8 * (t + 1) + 8],
                            in_=pt[:, 40 * k: 40 * k + 8])
                        nc.vector.tensor_copy(
                            out=HT[1][k][:, 8 * (t + 1): 8 * (t + 1) + 8],
                            in_=pt[:, 40 * k + 32: 40 * k + 40])
                    if t + 1 < S:
                        gx_add(t + 1, (3,))

                load_gx(0)
                gx_add(0, (0, 1, 2, 3))
                for _i in range(S):
                    body(_i)

        # ---------------- LSTM pass (shared for both layers) ------------------
        def lstm_pass(gx, wh, HT, tag):
            """Run fw+bw chains for one layer. gx/wh indexed by dir; results
            reshuffled into chunk-major HT[d][k] ([128, 8*(S+1)])."""
            c_t = [persist.tile([BL, H], f32, name=f"c{tag}{d}") for d in range(2)]
            hTq = [[persist.tile([128, BL], bf16, name=f"hTq{tag}{d}{k}")
                    for k in range(4)] for d in range(2)]
            for d in range(2):
                for k in range(4):
                    nc.vector.memset(hTq[d][k][:], 0.0)
            hTcm = persist.tile([128, 64], bf16, name=f"hTc{tag}")
            hTc = [hTcm[:, 0:32], hTcm[:, 32:64]]
            hts_cm = tc.tile_pool(name=f"hts{tag}", bufs=1)
            hts_pool = hts_cm.__enter__()
            HTS = hts_pool.tile([128, 2 * HCOLS], bf16, name=f"hts{tag}")
            nc.vector.memset(hTcm[:], 0.0)
            nc.vector.memset(HTS[:, 0:64], 0.0)
            for d in range(2):
                nc.vector.memset(c_t[d][:], 0.0)

            with tc.tile_pool(name=f"lp{tag}", bufs=2) as lp, \
                 tc.tile_pool(name=f"lpp{tag}", bufs=2, space="PSUM") as lpp:
                import os
                _BD = os.environ.get("BLSTM_BODY", "")
                gxv = [gx[d][:].rearrange("s b g -> (s b) g") for d in range(2)]
                def body(iv):
                    if isinstance(iv, int):
                        hoff = iv * 64 + 64
                        soff = iv * BL
                    else:
                        hoff = nc.vector.snap(iv * 64 + 64)
                        soff = nc.sync.snap(iv * BL)
                    for d in range(2):
                        gb = lp.tile([BL, G4], bf16, tag=f"gb{d}", bufs=3)
                        if isinstance(iv, int):
                            nc.sync.dma_start(gb[:], gxv[d][soff:soff + BL, :])
                        else:
                            nc.sync.dma_start(gb[:], gxv[d][ds(soff, BL), :])
                        act = {}
                        pbo = None
                        for n in (2, 0, 1, 3):      # f, i, j, o
                            pb = lpp.tile([BL, 512], f32, tag=f"pg{d}")
                            for k in range(4):
                                nc.tensor.matmul(
                                    out=pb[:],
                                    lhsT=hTq[d][k][:],
                                    rhs=wh[d][k][:, 512 * n: 512 * n + 512],
                                    start=(k == 0), stop=False)
                            nc.tensor.matmul(
                                out=pb[:], lhsT=id8[:],
                                rhs=gb[:, 512 * n: 512 * n + 512],
                                start=False, stop=True)
                            if n == 3:
                                pbo = pb
                                continue
                            a = lp.tile([BL, 512], f32, tag=f"act{d}{n}")
                            nc.scalar.activation(
                                out=a[:], in_=pb[:],
                                func=AF.Tanh if n == 1 else AF.Sigmoid)
                            act[n] = a
                        q = lp.tile([BL, H], f32, tag=f"qp{d}")
                        nc.vector.tensor_mul(out=q[:], in0=c_t[d][:], in1=act[2][:])
                        p = lp.tile([BL, H], f32, tag=f"qp{d}")
                        nc.vector.tensor_mul(out=p[:], in0=act[0][:], in1=act[1][:])
                        nc.vector.tensor_add(out=c_t[d][:], in0=q[:], in1=p[:])
                        tc_ = lp.tile([BL, H], f32, tag=f"tc{d}")
                        nc.scalar.activation(out=tc_[:], in_=c_t[d][:], func=AF.Tanh)
                        hn = lp.tile([BL, H], bf16, tag=f"hn{d}")
                        for k in range(4):
                            so = lp.tile([BL, 128], f32, tag=f"so{d}", bufs=3)
                            nc.scalar.activation(
                                out=so[:], in_=pbo[:, 128 * k: 128 * k + 128],
                                func=AF.Sigmoid)
                            nc.vector.tensor_mul(
                                out=hn[:, 128 * k: 128 * k + 128], in0=so[:],
                                in1=tc_[:, 128 * k: 128 * k + 128])
                            pt = lpp.tile([128, BL], bf16, tag=f"pt{d}")
                            nc.tensor.transpose(
                                out=pt[:], in_=hn[:, 128 * k: 128 * k + 128],
                                identity=id8[:])
                            nc.vector.tensor_copy(out=hTq[d][k][:], in_=pt[:])
                            nc.vector.tensor_copy(
                                out=hTcm[:, 32 * d + 8 * k: 32 * d + 8 * k + 8],
                                in_=pt[:])
                    if isinstance(iv, int):
                        nc.vector.tensor_copy(out=HTS[:, hoff:hoff + 64], in_=hTcm[:])
                    else:
                        nc.vector.tensor_copy(out=HTS[:, ds(hoff, 64)], in_=hTcm[:])
                import os as _os
                _UB = int(_os.environ.get("BLSTM_UB", "2"))
                if _os.environ.get("BLSTM_FULL", "0") == "1":
                    for _i in range(S):
                        body(_i)
                elif _os.environ.get("BLSTM_STAG", "1") == "1":
                    with tc.For_i(0, S, _UB, staggered_reset=True) as iv:
                        for _u in range(_UB):
                            body(iv + _u)
                else:
                    tc.For_i_unrolled(0, S, 1, body, max_unroll=2)
            sv = HTS[:].rearrange("p (s c) -> p s c", c=64)
            for d in range(2):
                for k in range(4):
                    nc.vector.tensor_copy(
                        out=HT[d][k][:].rearrange("p (s c) -> p s c", c=8),
                        in_=sv[:, :, 32 * d + 8 * k: 32 * d + 8 * k + 8])
            hts_cm.__exit__(None, None, None)

        def flip(HT_d, pool):
            """HF[k] step-block c = HT[k] step-block (S+1-c), c in 1..256."""
            W8 = 8 * (S + 1)
            HF = [pool.tile([128, W8], bf16, name=f"hf{id(HT_d)}_{k}")
                  for k in range(4)]
            for k in range(4):
                sv = HT_d[k][:].rearrange("p (c e) -> p c e", e=8)
                nc.vector.tensor_copy(
                    out=HF[k][:, 8:W8].rearrange("p (c e) -> p c e", e=8),
                    in_=sv[:, S:0:-1, :])
            return HF

        def hslice(Ht, k):
            return lambda m, _t=Ht[k]: _t[:, 128 * m + 8: 128 * m + 136]

        # ---------------- layer 0 ---------------------------------------------
        with tc.tile_pool(name="l0", bufs=1) as l0pool:
            HT0 = [[l0pool.tile([128, 8 * (S + 1)], bf16, name=f"ht0{d}_{k}")
                    for k in range(4)] for d in range(2)]
            with tc.tile_pool(name="l0w", bufs=1) as l0w:
                wh0 = [[l0w.tile([128, G4], bf16, name=f"wh0{d}_{k}")
                        for k in range(4)] for d in range(2)]
                for d in range(2):
                    for k in range(4):
                        o = E + 128 * k
                        nc.gpsimd.dma_start(wh0[d][k][:], w0_d[d, o:o + 128, :])
                (lstm_pass_v2 if _V2 else lstm_pass)(gx0, wh0, HT0, "a")
            if _BI <= 1:
                _fix_sync_waits(nc)
                return nc

            # ---------------- layer-1 input projections -----------------------
            with tc.tile_pool(name="ph2", bufs=1) as ph2, \
                 tc.tile_pool(name="ph2w", bufs=2) as ph2w, \
                 tc.tile_pool(name="ph2p", bufs=4, space="PSUM") as ph2p:
                HF0 = [flip(HT0[d], ph2) for d in range(2)]
                for d in range(2):
                    with tc.tile_pool(name=f"ph2x{d}", bufs=1) as ph2x:
                        wx1 = [ph2x.tile([128, G4], bf16, name=f"wx1{d}_{k}")
                               for k in range(8)]
                        for k in range(8):
                            nc.gpsimd.dma_start(wx1[k][:],
                                                w1_d[d, 128 * k: 128 * k + 128, :])
                        if d == 0:
                            lhs = [hslice(HT0[0], k) for k in range(4)] + \
                                  [hslice(HF0[1], k) for k in range(4)]
                        else:
                            lhs = [hslice(HF0[0], k) for k in range(4)] + \
                                  [hslice(HT0[1], k) for k in range(4)]
                        gemm_gates((gx1, d), lhs, wx1, b1_t[d], mb_t[d],
                                   ph2p, ph2w)
            if _BI <= 2:
                _fix_sync_waits(nc)
                return nc

        # ---------------- layer 1 ---------------------------------------------
        with tc.tile_pool(name="l1", bufs=1) as l1pool:
            HT1 = [[l1pool.tile([128, 8 * (S + 1)], bf16, name=f"ht1{d}_{k}")
                    for k in range(4)] for d in range(2)]
            with tc.tile_pool(name="l1w", bufs=1) as l1w:
                wh1 = [[l1w.tile([128, G4], bf16, name=f"wh1{d}_{k}")
                        for k in range(4)] for d in range(2)]
                for d in range(2):
                    for k in range(4):
                        o = 2 * H + 128 * k
                        nc.gpsimd.dma_start(wh1[d][k][:], w1_d[d, o:o + 128, :])
                (lstm_pass_v2 if _V2 else lstm_pass)(gx1, wh1, HT1, "b")
            if _BI <= 3:
                _fix_sync_waits(nc)
                return nc

            # ---------------- dense + softmax ---------------------------------
            with tc.tile_pool(name="dn", bufs=3) as dn, \
                 tc.tile_pool(name="dnp", bufs=3, space="PSUM") as dnp:
                HF1b = flip(HT1[1], dn)
                wdt = [dn.tile([128, C], bf16, name=f"wdt{k}", tag=f"wd{k}") for k in range(8)]
                for k in range(8):
                    nc.gpsimd.dma_start(wdt[k][:], wd_d[128 * k: 128 * k + 128, :])
                bdt = dn.tile([1, C], bf16, tag="bd")
                nc.gpsimd.dma_start(bdt[:], bd_d[:].rearrange("(o c) -> o c", o=1))
                lhs = [hslice(HT1[0], k) for k in range(4)] + \
                      [hslice(HF1b, k) for k in range(4)]
                for m in range(NT):
                    pb = dnp.tile([128, C], f32, tag="dps")
                    nc.tensor.matmul(out=pb[:], lhsT=ones1[:], rhs=bdt[:],
                                     start=True, stop=False)
                    for k in range(8):
                        nc.tensor.matmul(
                            out=pb[:], lhsT=lhs[k](m),
                            rhs=wdt[k][:], start=False, stop=(k == 7))
                    mx = dn.tile([128, 1], f32, tag="dmx")
                    nc.vector.tensor_reduce(out=mx[:], in_=pb[:],
                                            axis=mybir.AxisListType.X,
                                            op=OP.max, negate=True)
                    ex = dn.tile([128, C], f32, tag="dex")
                    ssum = dn.tile([128, 1], f32, tag="dsum")
                    nc.scalar.activation(out=ex[:], in_=pb[:], func=AF.Exp,
                                         bias=mx[:, 0:1], accum_out=ssum[:, 0:1])
                    rinv = dn.tile([128, 1], f32, tag="drinv")
                    nc.vector.reciprocal(out=rinv[:], in_=ssum[:, 0:1])
                    ot = dn.tile([128, C], f32, tag="dout")
                    nc.vector.tensor_scalar_mul(out=ot[:], in0=ex[:],
                                                scalar1=rinv[:, 0:1])
                    nc.sync.dma_start(out_d[128 * m: 128 * m + 128, :], ot[:])

    _fix_sync_waits(nc)
    return nc


def pack_in_maps(input_ids, lengths, emb, w_fw0, b_fw0, w_bw0, b_bw0,
                 w_fw1, b_fw1, w_bw1, b_bw1, wd, bd):
    import os
    _v2 = os.environ.get("BLSTM_V2", "1") == "1"
    input_ids = np.asarray(input_ids)
    lengths = np.asarray(lengths)
    f = np.asarray

    def gw(w):
        """Reorder gate blocks i,j,f,o -> j,i,f,o for the v2 pass."""
        w = f(w, dtype=np.float32)
        if not _v2:
            return w
        hh = w.shape[-1] // 4
        blocks = [w[..., i * hh:(i + 1) * hh] for i in range(4)]
        return np.concatenate([blocks[1], blocks[0], blocks[2], blocks[3]], axis=-1)

    gate_cols = (1, 0, 2, 3) if _v2 else (0, 1, 2, 3)  # new n -> old gate idx
    in_maps = []
    for c in range(NCORES):
        rows = slice(c * BL, (c + 1) * BL)
        ids_s = np.ascontiguousarray(input_ids[rows])          # [BL, S]
        len_s = lengths[rows]                                  # [BL]
        ids_fw = ids_s.T.reshape(-1)                           # token j*BL+b -> ids[b, j]
        ids_bw = ids_s[:, ::-1].T.reshape(-1)                  # -> ids[b, S-1-j]
        j = np.arange(S)[:, None]                              # [S,1]
        m_fw = (j < len_s[None, :]).astype(np.float32)         # [S, BL]
        m_bw = ((S - 1 - j) < len_s[None, :]).astype(np.float32)
        mb2 = np.zeros((2, S, BL, 4), np.float32)
        for d, m in enumerate((m_fw, m_bw)):
            inv = 1.0 - m
            gv = {0: -MASK_BIG * inv, 1: 0.0, 2: 1.0 + MASK_BIG * inv,
                  3: -MASK_BIG * inv}                          # old order i,j,f,o
            for n in range(4):
                mb2[d, :, :, n] = gv[gate_cols[n]]
        in_maps.append({
            "emb": np.ascontiguousarray(f(emb, dtype=np.float32)),
            "ids2": np.stack([ids_fw, ids_bw]).astype(np.int32),
            "mb2": mb2.reshape(2, T, 4),
            "w0": np.stack([gw(w_fw0), gw(w_bw0)]),
            "b0": np.stack([gw(b_fw0), gw(b_bw0)]),
            "w1": np.stack([gw(w_fw1), gw(w_bw1)]),
            "b1": np.stack([gw(b_fw1), gw(b_bw1)]),
            "wd": f(wd, dtype=np.float32),
            "bd": f(bd, dtype=np.float32),
        })
    return in_maps


def kernel(input_ids, lengths, emb, w_fw0, b_fw0, w_bw0, b_bw0,
           w_fw1, b_fw1, w_bw1, b_bw1, wd, bd):
    global _compiled
    from concourse.bass_utils import run_bass_kernel_spmd

    if _compiled is None:
        _compiled = _build()
    nc = _compiled

    in_maps = pack_in_maps(input_ids, lengths, emb, w_fw0, b_fw0, w_bw0, b_bw0,
                           w_fw1, b_fw1, w_bw1, b_bw1, wd, bd)
    global _last_in_maps
    _last_in_maps = in_maps
    res = run_bass_kernel_spmd(nc, in_maps, core_ids=list(range(NCORES)))
    out = np.zeros((B, S, C), np.float32)
    for c in range(NCORES):
        out[c * BL:(c + 1) * BL] = (
            res.results[c]["out"].reshape(S, BL, C).transpose(1, 0, 2))
    return out

